# revision 1
# baseline (speedup 1.0000x reference)
"""VQ codebook encoding (nn_Encoding) Trainium2 Bass kernel.

Math (per batch b):
  Xf = X[b].reshape(D, N).T                      # [N, D], N = H*W
  SL[n,k] = scale[k] * (||x_n||^2 - 2 x_n.c_k + ||c_k||^2)
  A = softmax_k(SL)                              # no max-subtraction needed (|SL| < ~50)
  E[b,k,:] = sum_n A[n,k] * x_n  -  (sum_n A[n,k]) * c_k

Sharding: data-parallel over B: 16 batches -> 2 per NeuronCore x 8 cores.
No collectives needed; outputs are concatenated on the host.

Device pipeline per batch (all matmuls bf16, softmax math fp32):
  - M1 (PE):  SL^T chunks [128n, 64k] = Xd-tile-stationary matmuls vs (-2*scale*C)^T,
              plus a rank-1 aug matmul adding scale*(c2-1) (the -1 compensates the
              ones column folded into the squared-norm below).
  - x2 (ACT/DVE): ||x_n||^2 + 1 via Square+accum / tensor_tensor_reduce over the
              [N, 257] host-transposed X (last column = ones, reused by M2).
  - softmax:  expin = scale_k*x2'_n + SL (DVE scalar_tensor_tensor, PSUM src),
              exp (ACT, batched 512 wide), Z row-sums (DVE reduce), reciprocal (DVE),
              A = expS * Zinv (GPSIMD tensor_scalar, bf16).
  - M2 (PE):  [E1 | asum] [64, 257] += A_chunk^T-stationary @ [X^T | ones] moving,
              accumulated over all 72 chunks in one PSUM bank.
  - E = E1 - asum * C  (DVE scalar_tensor_tensor), DMA out fp32.
"""

import numpy as np

import concourse.bacc as bacc
import concourse.mybir as mybir
from concourse.bass_utils import run_bass_kernel_spmd
from concourse.tile import TileContext

# Problem constants (hardcoded per harness contract)
B, D, HH, WW = 16, 256, 96, 96
K = 64
N = HH * WW              # 9216
NC = 8                   # cores
NB = B // NC             # batches per core = 2
NCHUNK = N // 128        # 72 chunks of 128 spatial positions
G = 4                    # chunks per softmax group
NGROUP = NCHUNK // G     # 9 groups

F32 = mybir.dt.float32
BF16 = mybir.dt.bfloat16
NP_BF16 = mybir.dt.np(BF16)

_STATE = {}

# Bisection/er tuning knobs
OPTS = {
    "a_engine": "gpsimd",   # "gpsimd" | "vector": engine for A = expS * Zinv
    "do_x2": True,           # compute squared norms (else constant scalar)
    "do_m1": True,           # distance matmuls
    "do_m2": True,           # aggregation matmul + E finalize
    "do_softmax": True,      # softmax chain (exp etc.)
}


def _build_nc(loop_n=None):
    """loop_n: if set, wrap the whole computation in a For_i repeat loop
    (benchmark variant — measures steady-state HW time per iteration)."""
    nc = bacc.Bacc("TRN2", target_bir_lowering=False, debug=False)

    # DRAM I/O (per-core shard)
    xd = nc.dram_tensor("xd", [NB, 128, 2 * N], BF16, kind="ExternalInput").ap()
    xto = nc.dram_tensor("xto", [NB, 128, NCHUNK * 257], BF16, kind="ExternalInput").ap()
    cm = nc.dram_tensor("cm", [128, 2 * K], BF16, kind="ExternalInput").ap()
    sc2 = nc.dram_tensor("sc2", [1, K], BF16, kind="ExternalInput").ap()
    ones = nc.dram_tensor("ones", [1, 128], BF16, kind="ExternalInput").ap()
    scalet = nc.dram_tensor("scalet", [128, K], F32, kind="ExternalInput").ap()
    cw = nc.dram_tensor("cw", [K, D], F32, kind="ExternalInput").ap()
    e_out = nc.dram_tensor("e", [NB, K, D], F32, kind="ExternalOutput").ap()

    AF = mybir.ActivationFunctionType
    OP = mybir.AluOpType
    AX = mybir.AxisListType

    with TileContext(nc) as tc:
        with (
            tc.tile_pool(name="const", bufs=1) as constp,
            tc.tile_pool(name="xd", bufs=2) as xdp,
            tc.tile_pool(name="xto", bufs=2) as xtop,
            tc.tile_pool(name="work", bufs=4) as workp,
            tc.tile_pool(name="sq", bufs=8) as sqp,
            tc.tile_pool(name="out", bufs=2) as outp,
            tc.tile_pool(name="psl", bufs=4, space="PSUM") as pslp,
            tc.tile_pool(name="pe", bufs=4, space="PSUM") as pep,
        ):
            cm_sb = constp.tile([128, 2 * K], BF16)
            sc2_sb = constp.tile([1, K], BF16)
            ones_sb = constp.tile([1, 128], BF16)
            scale_sb = constp.tile([128, K], F32)
            cw_sb = constp.tile([K, D], F32)
            nc.sync.dma_start(out=cm_sb[:], in_=cm[:])
            nc.sync.dma_start(out=sc2_sb[:], in_=sc2[:])
            nc.sync.dma_start(out=ones_sb[:], in_=ones[:])
            nc.sync.dma_start(out=scale_sb[:], in_=scalet[:])
            nc.sync.dma_start(out=cw_sb[:], in_=cw[:])

            import contextlib
            hints = (mybir.EngineType.PE, mybir.EngineType.DVE,
                     mybir.EngineType.Activation, mybir.EngineType.Pool,
                     mybir.EngineType.SP)
            loop_ctx = (tc.For_i(0, loop_n, 1, hint_engines=hints) if loop_n
                        else contextlib.nullcontext())
            with loop_ctx:
                _kernel_body(nc, tc, locals())

    nc.compile()
    return nc


def _kernel_body(nc, tc, env):
    xd, xto, e_out = env["xd"], env["xto"], env["e_out"]
    xdp, xtop, workp, sqp, outp = (env["xdp"], env["xtop"], env["workp"],
                                   env["sqp"], env["outp"])
    pslp, pep = env["pslp"], env["pep"]
    cm_sb, sc2_sb, ones_sb, scale_sb, cw_sb = (
        env["cm_sb"], env["sc2_sb"], env["ones_sb"], env["scale_sb"], env["cw_sb"])
    AF = mybir.ActivationFunctionType
    OP = mybir.AluOpType
    AX = mybir.AxisListType
    NQ = 8                      # DMA split: overlap load with compute
    NQC = NCHUNK // NQ          # chunks covered per slice
    for b in range(NB):
        xd_sb = xdp.tile([128, 2 * N], BF16, tag="xd")
        xto_sb = xtop.tile([128, NCHUNK * 257], BF16, tag="xto")
        xdv_s = xd_sb[:].rearrange("p (t n) -> p t n", t=2)
        xdv_d = xd[b].rearrange("p (t n) -> p t n", t=2)
        for q in range(NQ):
            n0, n1 = q * NQC * 128, (q + 1) * NQC * 128
            nc.sync.dma_start(out=xdv_s[:, :, n0:n1], in_=xdv_d[:, :, n0:n1])
            c0, c1 = q * NQC * 257, (q + 1) * NQC * 257
            nc.sync.dma_start(out=xto_sb[:, c0:c1], in_=xto[b][:, c0:c1])

        psum_e = pep.tile([K, 257], F32, tag="pe", name="psum_e") if OPTS["do_m2"] else None

        for g in range(NGROUP):
            psum_sl = pslp.tile([128, G * K], F32, tag="psl")
            x2g = workp.tile([128, G], F32, tag="x2g")
            expin = workp.tile([128, G * K], F32, tag="expin")
            expS = workp.tile([128, G * K], BF16, tag="expS")
            zg = workp.tile([128, G], F32, tag="zg")

            zinv_b = workp.tile([128, G], BF16, tag="zinvb")
            a_sb = workp.tile([128, G * K], BF16, tag="a")

            for j in range(G):
                c = g * G + j
                xto_c = xto_sb[:, c * 257:(c + 1) * 257]
                # squared norms (+1 from the ones column), fp32 accum
                if OPTS["do_x2"]:
                    if OPTS.get("x2_light"):
                        if j == 0:
                            nc.vector.memset(x2g[:], 1.0)
                    elif j in (0, 3, 6):  # 3/8 on ACT, 5/8 on DVE
                        sq_a = sqp.tile([128, 257], BF16, tag="sq_a")
                        nc.scalar.activation(
                            sq_a[:], xto_c, AF.Square,
                            accum_out=x2g[:, j:j + 1],
                        )
                    else:
                        # NOTE: tensor_tensor_reduce hangs on this HW stack;
                        # scalar_tensor_tensor with accum_out is equivalent:
                        # out = (x * 1) * x, accum = sum(out)
                        sq_d = sqp.tile([128, 257], BF16, tag="sq_d")
                        nc.vector.scalar_tensor_tensor(
                            out=sq_d[:], in0=xto_c, scalar=1.0, in1=xto_c,
                            op0=OP.mult, op1=OP.mult,
                            accum_out=x2g[:, j:j + 1],
                        )
                # M1: SL^T chunk [128n, 64k]
                out_sl = psum_sl[:, j * K:(j + 1) * K]
                if OPTS["do_m1"]:
                    nc.tensor.matmul(
                        out_sl, lhsT=xd_sb[:, c * 128:(c + 1) * 128],
                        rhs=cm_sb[:, 0:K], start=True, stop=False)
                    nc.tensor.matmul(
                        out_sl, lhsT=xd_sb[:, N + c * 128:N + (c + 1) * 128],
                        rhs=cm_sb[:, K:2 * K], start=False, stop=False)
                    nc.tensor.matmul(
                        out_sl, lhsT=ones_sb[:], rhs=sc2_sb[:],
                        start=False, stop=True)
                else:
                    nc.tensor.matmul(
                        out_sl, lhsT=ones_sb[:], rhs=sc2_sb[:],
                        start=True, stop=True)
            if OPTS["do_softmax"]:
                # W = scale_k * x2'_n  (one batched op per group, gpsimd)
                x2b = x2g[:].to_broadcast((128, G, K))
                scale_rep = scale_sb[:].rearrange(
                    "p (o k) -> p o k", o=1).to_broadcast((128, G, K))
                w_eng = nc.gpsimd if OPTS["a_engine"] == "gpsimd" else nc.vector
                ev = expin[:].rearrange("p (g k) -> p g k", g=G)
                w_eng.tensor_tensor(out=ev, in0=x2b, in1=scale_rep, op=OP.mult)
                # expin += SL (from PSUM)
                nc.vector.tensor_tensor(out=expin[:], in0=expin[:],
                                        in1=psum_sl[:], op=OP.add)
                nc.scalar.activation(expS[:], expin[:], AF.Exp)
                nc.vector.tensor_reduce(
                    out=zg[:], in_=expS[:].rearrange("p (g k) -> p g k", g=G),
                    axis=AX.X, op=OP.add,
                )
                with nc.allow_low_precision(reason="zinv bf16 for A-mult"):
                    nc.vector.reciprocal(zinv_b[:], zg[:])
                # A = expS * (1/Z)  (one batched op per group)
                av = a_sb[:].rearrange("p (g k) -> p g k", g=G)
                esv = expS[:].rearrange("p (g k) -> p g k", g=G)
                w_eng.tensor_tensor(out=av, in0=esv,
                                    in1=zinv_b[:].to_broadcast((128, G, K)),
                                    op=OP.mult)
            else:
                nc.vector.tensor_copy(a_sb[:], xto_sb[:, g * 512:(g + 1) * 512])

            if OPTS["do_m2"]:
                for j in range(G):
                    c = g * G + j
                    nc.tensor.matmul(
                        psum_e[:], lhsT=a_sb[:, j * K:(j + 1) * K],
                        rhs=xto_sb[:, c * 257:(c + 1) * 257],
                        start=(c == 0), stop=(c == NCHUNK - 1),
                    )

        # E = E1 - asum * C
        if OPTS["do_m2"]:
            nasum = outp.tile([K, 1], F32, tag="nasum")
            nc.vector.tensor_scalar(
                out=nasum[:], in0=psum_e[:, 256:257],
                scalar1=-1.0, scalar2=None, op0=OP.mult,
            )
            e_sb = outp.tile([K, D], F32, tag="e_sb")
            nc.vector.scalar_tensor_tensor(
                out=e_sb[:], in0=cw_sb[:], scalar=nasum[:],
                in1=psum_e[:, 0:D], op0=OP.mult, op1=OP.add,
            )
        else:
            e_sb = outp.tile([K, D], F32, tag="e_sb")
            nc.vector.tensor_copy(e_sb[:], a_sb[0:K, 0:D])
        nc.sync.dma_start(out=e_out[b], in_=e_sb[:])


def _get_nc(loop_n=None):
    key = ("nc", loop_n)
    if key not in _STATE:
        _STATE[key] = _build_nc(loop_n)
    return _STATE[key]


def _prep_shared(codewords, scale):
    c2 = (codewords.astype(np.float64) ** 2).sum(1)
    cm_f = (-2.0 * scale[:, None] * codewords).T          # [D, K]
    cm_host = np.ascontiguousarray(
        np.concatenate([cm_f[0:128], cm_f[128:256]], axis=1)
    ).astype(NP_BF16)                                      # [128, 2K]
    sc2_host = (scale * (c2 - 1.0)).astype(np.float32)[None, :].astype(NP_BF16)
    ones_host = np.ones((1, 128), NP_BF16)
    scalet_host = np.ascontiguousarray(
        np.broadcast_to(scale.astype(np.float32)[None, :], (128, K))
    )
    cw_host = np.ascontiguousarray(codewords.astype(np.float32))
    return cm_host, sc2_host, ones_host, scalet_host, cw_host


def _prep_core(Xcore):
    """Xcore: [NB, D, H, W] fp32 -> (xd, xto) bf16 device layouts."""
    nb = Xcore.shape[0]
    Xf = Xcore.reshape(nb, D, N)
    Xbf = Xf.astype(NP_BF16)
    # xd: [nb, 128, 2N]; [b, p, t*N + n] = X[b, t*128+p, n]
    xd = np.ascontiguousarray(
        Xbf.reshape(nb, 2, 128, N).transpose(0, 2, 1, 3).reshape(nb, 128, 2 * N)
    )
    # xto: [nb, 128, 72*257]; chunk c holds [X^T rows c*128+p | 1.0]
    XT = np.ascontiguousarray(Xf.transpose(0, 2, 1)).astype(NP_BF16)  # [nb, N, D]
    XTO = np.concatenate([XT, np.ones((nb, N, 1), NP_BF16)], axis=2)  # [nb, N, 257]
    xto = np.ascontiguousarray(
        XTO.reshape(nb, NCHUNK, 128, 257).transpose(0, 2, 1, 3).reshape(nb, 128, NCHUNK * 257)
    )
    return xd, xto


def run(X, codewords, scale, trace=False):
    X = np.asarray(X, np.float32)
    codewords = np.asarray(codewords, np.float32)
    scale = np.asarray(scale, np.float32)
    nc = _get_nc()
    cm_host, sc2_host, ones_host, scalet_host, cw_host = _prep_shared(codewords, scale)
    in_maps = []
    for i in range(NC):
        xd_i, xto_i = _prep_core(X[i * NB:(i + 1) * NB])
        in_maps.append({
            "xd": xd_i, "xto": xto_i, "cm": cm_host, "sc2": sc2_host,
            "ones": ones_host, "scalet": scalet_host, "cw": cw_host,
        })
    res = run_bass_kernel_spmd(nc, in_maps, list(range(NC)), trace=trace)
    E = np.empty((B, K, D), np.float32)
    for i in range(NC):
        E[i * NB:(i + 1) * NB] = res.results[i]["e"]
    return E, res


def kernel(X, codewords, scale):
    E, _ = run(X, codewords, scale)
    return E



# revision 14
# speedup vs baseline: 1.0240x; 1.0240x over previous
"""VQ codebook encoding (nn_Encoding) Trainium2 Bass kernel — v2.

Math (per batch b):
  Xf = X[b].reshape(D, N).T                      # [N, D], N = H*W
  SL[n,k] = scale[k] * (||x_n||^2 - 2 x_n.c_k + ||c_k||^2)
  A = softmax_k(SL)                              # no max-subtraction needed (|SL| < ~50)
  E[b,k,:] = sum_n A[n,k] * x_n  -  (sum_n A[n,k]) * c_k

Sharding: data-parallel over B: 16 batches -> 2 per NeuronCore x 8 cores.
No collectives; outputs are concatenated on the host.

v2 design (vs v1 baseline at ~102us):
  - M1 cross term on PE in fp8e4 (DoubleRow, both 128-d halves contracted in
    one matmul, cm prescaled by 2^8 against fp8 subnormals; exp applies 2^-8).
    Halves the xd DMA traffic and 3x fewer PE instructions.
  - The whole pre-exp chain (scale_k*x2_n + scale_k*(c2_k+256)) is folded into
    ONE small fp32r "aug" matmul per 4-chunk group: lhsT = [x2c rows; ones]
    (from a PE transpose of the accumulated x2 columns), rhs = a host-built
    block matrix [scale blocks; sc2 row]. Replaces W-mult + expin-add + rank-1.
  - exp reads PSUM directly on ACT with scale=2^-8; squares (x2) are split
    across ACT/DVE/Pool per a balance pattern; Z-reduce / A-mult engine
    selectable.
  - x2 is accumulated per 16-chunk "wave" into columns of a [128,128] tile,
    transposed once per wave on PE (identity matmul) so the aug lhsT rows land
    at partition offsets {0,32,64,96} (valid PE tile positions).
"""

import numpy as np

import concourse.bacc as bacc
import concourse.mybir as mybir
from concourse.bass_utils import run_bass_kernel_spmd
from concourse.masks import make_identity
from concourse.tile import TileContext

# Problem constants (hardcoded per harness contract)
B, D, HH, WW = 16, 256, 96, 96
K = 64
N = HH * WW              # 9216
NC = 8                   # cores
NB = B // NC             # batches per core = 2
NCHUNK = N // 128        # 72 chunks of 128 spatial positions
G = 8                    # chunks per group (one full 2KB PSUM bank of SL)
NGROUP = NCHUNK // G     # 9 groups
WAVE_G = 3               # groups per x2-transpose wave (base partitions 0/32/64)
NWAVE = (NGROUP + WAVE_G - 1) // WAVE_G  # 3 waves of 3 groups
SEXP = 8                 # fp8/aug prescale 2^SEXP, exp applies 2^-SEXP

F32 = mybir.dt.float32
F32R = mybir.dt.float32r
BF16 = mybir.dt.bfloat16
FP8 = mybir.dt.float8e4
NP_BF16 = mybir.dt.np(BF16)
NP_FP8 = mybir.dt.np(FP8)

_STATE = {}

# Tuning knobs
OPTS = {
    # squares engine pattern, cycled over chunks: A=ACT, V=DVE, P=Pool/gpsimd
    "sq_pattern": "AVVAVVAV",
    "z_engine": "vector",    # Z row-sum engine (gpsimd lacks free-axis reduce)
    "a_engine": "gpsimd",    # A = expS * Zinv engine
}


def _build_nc(loop_n=None):
    """loop_n: if set, wrap the whole computation in a For_i repeat loop
    (benchmark variant — measures steady-state HW time per iteration)."""
    nc = bacc.Bacc("TRN2", target_bir_lowering=False, debug=False)

    # DRAM I/O (per-core shard)
    xd8 = nc.dram_tensor("xd8", [NB, 128, 2 * N], FP8, kind="ExternalInput").ap()
    xto = nc.dram_tensor("xto", [NB, 128, NCHUNK * 257], BF16, kind="ExternalInput").ap()
    cm8 = nc.dram_tensor("cm8", [128, 2 * K], FP8, kind="ExternalInput").ap()
    augmove = nc.dram_tensor("augmove", [128, G * K], F32R, kind="ExternalInput").ap()
    cw = nc.dram_tensor("cw", [K, D], F32, kind="ExternalInput").ap()
    e_out = nc.dram_tensor("e", [NB, K, D], F32, kind="ExternalOutput").ap()

    with TileContext(nc) as tc:
        with (
            tc.tile_pool(name="const", bufs=1) as constp,
            tc.tile_pool(name="xd", bufs=2) as xdp,
            tc.tile_pool(name="xto", bufs=2) as xtop,
            tc.tile_pool(name="work", bufs=4) as workp,
            tc.tile_pool(name="sq", bufs=6) as sqp,
            tc.tile_pool(name="wave", bufs=3) as wavep,
            tc.tile_pool(name="augt", bufs=3) as augp,
            tc.tile_pool(name="out", bufs=2) as outp,
            tc.tile_pool(name="psl", bufs=3, space="PSUM") as pslp,
            tc.tile_pool(name="pe", bufs=2, space="PSUM") as pep,
            tc.tile_pool(name="pt", bufs=2, space="PSUM") as ptp,
        ):
            cm8_sb = constp.tile([128, 2 * K], FP8)
            augmove_sb = constp.tile([128, G * K], F32R)
            cw_sb = constp.tile([K, D], F32)
            ident = constp.tile([128, 128], F32)
            nc.sync.dma_start(out=cm8_sb[:], in_=cm8[:])
            nc.sync.dma_start(out=augmove_sb[:], in_=augmove[:])
            nc.sync.dma_start(out=cw_sb[:], in_=cw[:])
            make_identity(nc, ident[:])

            import contextlib
            hints = (mybir.EngineType.PE, mybir.EngineType.DVE,
                     mybir.EngineType.Activation, mybir.EngineType.Pool,
                     mybir.EngineType.SP)
            loop_ctx = (tc.For_i(0, loop_n, 1, hint_engines=hints) if loop_n
                        else contextlib.nullcontext())
            with loop_ctx:
                _kernel_body(nc, tc, locals())

    nc.compile()
    return nc


def _kernel_body(nc, tc, env):
    xd8, xto, e_out = env["xd8"], env["xto"], env["e_out"]
    xdp, xtop, workp, sqp, wavep, augp, outp = (
        env["xdp"], env["xtop"], env["workp"], env["sqp"], env["wavep"],
        env["augp"], env["outp"])
    pslp, pep, ptp = env["pslp"], env["pep"], env["ptp"]
    cm8_sb, augmove_sb, cw_sb, ident = (
        env["cm8_sb"], env["augmove_sb"], env["cw_sb"], env["ident"])
    AF = mybir.ActivationFunctionType
    OP = mybir.AluOpType
    AX = mybir.AxisListType
    DR = mybir.MatmulPerfMode.DoubleRow
    sq_pat = OPTS["sq_pattern"]
    z_eng = nc.vector if OPTS["z_engine"] == "vector" else nc.gpsimd
    a_eng = nc.vector if OPTS["a_engine"] == "vector" else nc.gpsimd

    NQ = 8                      # DMA split: overlap load with compute
    NQC = NCHUNK // NQ          # chunks covered per slice

    cm8v = cm8_sb[:].rearrange("p (t k) -> p t k", t=2)

    # Allocate both batches' input tiles up front and queue all DMA slices
    # (pools have bufs=2, so in the For_i steady state slice loads overlap the
    # previous iteration's tail compute).
    xd8vs, xto_sbs = [], []
    for b in range(NB):
        xd8_sb = xdp.tile([128, 2 * N], FP8, tag="xd8", name="xd8_sb")
        xto_sb = xtop.tile([128, NCHUNK * 257], BF16, tag="xto", name="xto_sb")
        xd8vs.append(xd8_sb[:].rearrange("p (t n) -> p t n", t=2))
        xto_sbs.append(xto_sb[:])
    for q in range(NQ):
        for b in range(NB):
            xd8v_d = xd8[b].rearrange("p (t n) -> p t n", t=2)
            c0, c1 = q * NQC * 257, (q + 1) * NQC * 257
            nc.sync.dma_start(out=xto_sbs[b][:, c0:c1], in_=xto[b][:, c0:c1])
            n0, n1 = q * NQC * 128, (q + 1) * NQC * 128
            nc.sync.dma_start(out=xd8vs[b][:, :, n0:n1], in_=xd8v_d[:, :, n0:n1])

    psum_es = {}
    x2ws, augTs = {}, {}

    def ensure_x2w(b, w):
        if (b, w) in x2ws:
            return x2ws[(b, w)]
        nw = min(WAVE_G * (w + 1), NGROUP) - WAVE_G * w
        x2w = wavep.tile([128, 128], F32, tag="x2w", name="x2w")
        onesv = x2w[:].rearrange("p (a c) -> p a c", c=32)[:, 0:nw, G:G + 1]
        # 258 so the uniform -257 shift in emit_transpose leaves exactly 1.0
        nc.vector.memset(onesv, 258.0)
        x2ws[(b, w)] = x2w
        return x2w

    def emit_square_chunk(b, c):
        g = c // G
        w, m, j = g // WAVE_G, g % WAVE_G, c % G
        x2w = ensure_x2w(b, w)
        xto_c = xto_sbs[b][:, c * 257:(c + 1) * 257]
        col = 32 * m + j
        eng = sq_pat[c % len(sq_pat)]
        if eng == "A":
            sq_a = sqp.tile([128, 257], F32, tag="sq_a", name="sq_a")
            nc.scalar.activation(sq_a[:], xto_c, AF.Square,
                                 accum_out=x2w[:, col:col + 1])
        else:
            sq_d = sqp.tile([128, 257], F32, tag="sq_d", name="sq_d")
            nc.vector.scalar_tensor_tensor(
                out=sq_d[:], in0=xto_c, scalar=1.0, in1=xto_c,
                op0=OP.mult, op1=OP.mult,
                accum_out=x2w[:, col:col + 1])

    def emit_transpose(b, w):
        psum_t = ptp.tile([128, 128], F32, tag="pt", name="psum_t")
        nc.tensor.transpose(psum_t[:], x2ws[(b, w)][:], ident[:])
        augT = augp.tile([128, 128], F32R, tag="augT", name="augT")
        # x2 columns accumulate sum(x^2)+1 (ones col of xto); recenter by
        # -257 here so the fp32r aug matmul multiplies small values (tf32
        # mantissa) -- the 256 offset is folded into sc2 on the host.
        nc.vector.tensor_scalar(
            out=augT[:], in0=psum_t[:], scalar1=-257.0, scalar2=None,
            op0=OP.add)
        augTs[(b, w)] = augT

    def emit_m1(b, g):
        m = g % WAVE_G
        psl = pslp.tile([128, 512], F32, tag="psl", name="psl")
        p0 = 32 * m
        augT = augTs[(b, g // WAVE_G)]
        nc.tensor.matmul(
            psl[:, 0:G * K], lhsT=augT[p0:p0 + G + 1, :],
            rhs=augmove_sb[p0:p0 + G + 1, :], start=True, stop=False)
        for j in range(G):
            c = g * G + j
            nc.tensor.matmul(
                psl[:, j * K:(j + 1) * K],
                lhsT=xd8vs[b][:, :, c * 128:(c + 1) * 128],
                rhs=cm8v[:, :, :],
                start=False, stop=(j == G - 1), perf_mode=DR)
        return psl

    def emit_vec(g, psl, sq_thunks=()):
        """Vector chain for group g, with next-wave square ops interleaved
        between the cross-engine-dependent steps so no in-order engine queue
        stalls long on an upstream dependency."""
        sq = list(sq_thunks)

        def drip(k):
            for _ in range(k):
                if sq:
                    sq.pop(0)()

        drip(2)
        expS = workp.tile([128, G * K], BF16, tag="expS", name="expS")
        nc.scalar.activation(expS[:], psl[:, 0:G * K], AF.Exp,
                             scale=float(2.0 ** -SEXP))
        drip(1)
        zg = workp.tile([128, G], F32, tag="zg", name="zg")
        z_eng.tensor_reduce(
            out=zg[:], in_=expS[:].rearrange("p (g k) -> p g k", g=G),
            axis=AX.X, op=OP.add)
        drip(1)
        zinv = workp.tile([128, G], BF16, tag="zinv", name="zinv")
        with nc.allow_low_precision(reason="zinv bf16 for A-mult"):
            nc.vector.reciprocal(zinv[:], zg[:])
        a_sb = workp.tile([128, G * K], BF16, tag="a", name="a_sb")
        av = a_sb[:].rearrange("p (g k) -> p g k", g=G)
        esv = expS[:].rearrange("p (g k) -> p g k", g=G)
        a_eng.tensor_tensor(out=av, in0=esv,
                            in1=zinv[:].to_broadcast((128, G, K)),
                            op=OP.mult)
        drip(len(sq))
        return a_sb

    def emit_finalize(b):
        psum_e = psum_es[b]
        nasum = outp.tile([K, 1], F32, tag="nasum", name="nasum")
        nc.vector.tensor_scalar(
            out=nasum[:], in0=psum_e[:, 256:257],
            scalar1=-1.0, scalar2=None, op0=OP.mult)
        e_sb = outp.tile([K, D], F32, tag="e_sb", name="e_sb")
        nc.vector.scalar_tensor_tensor(
            out=e_sb[:], in0=cw_sb[:], scalar=nasum[:],
            in1=psum_e[:, 0:D], op0=OP.mult, op1=OP.add)
        nc.sync.dma_start(out=e_out[b], in_=e_sb[:])

    def emit_m2(b, g, a_sb):
        if b not in psum_es:
            psum_es[b] = pep.tile([K, 257], F32, tag="pe", name="psum_e")
        for j in range(G):
            c = g * G + j
            nc.tensor.matmul(
                psum_es[b][:], lhsT=a_sb[:, j * K:(j + 1) * K],
                rhs=xto_sbs[b][:, c * 257:(c + 1) * 257],
                start=(c == 0), stop=(c == NCHUNK - 1))
        if g == NGROUP - 1:
            emit_finalize(b)

    # Startup: wave (0,0) squares + transpose before the main pipeline.
    for c in range(0, WAVE_G * G):
        emit_square_chunk(0, c)
    emit_transpose(0, 0)

    # Main pipeline over all (b, g). While processing wave w of batch b, the
    # squares of the NEXT wave are emitted in thirds after each group's vector
    # chain (so exp never queues behind a long run of squares), and the next
    # wave's PE transpose goes out at the wave boundary.
    pending = None
    for b in range(NB):
        for g in range(NGROUP):
            w = g // WAVE_G
            if w + 1 < NWAVE:
                nxt = (b, w + 1)
            elif b + 1 < NB:
                nxt = (b + 1, 0)
            else:
                nxt = None
            psl = emit_m1(b, g)
            if pending is not None:
                emit_m2(*pending)
            sq_thunks = []
            if nxt is not None:
                nb_, nw_ = nxt
                base = nw_ * WAVE_G * G
                part = g % WAVE_G
                sq_thunks = [
                    (lambda bb, cc: (lambda: emit_square_chunk(bb, cc)))(nb_, c)
                    for c in range(base + part * G, base + (part + 1) * G)
                ]
            a_sb = emit_vec(g, psl, sq_thunks)
            pending = (b, g, a_sb)
            if nxt is not None and g % WAVE_G == WAVE_G - 1:
                emit_transpose(*nxt)
    emit_m2(*pending)


def _get_nc(loop_n=None):
    key = ("nc", loop_n)
    if key not in _STATE:
        _STATE[key] = _build_nc(loop_n)
    return _STATE[key]


def _prep_shared(codewords, scale):
    """Host-side constants: cm8 (fp8, 2^SEXP prescale), augmove (fp32),
    cw (fp32 codewords for the E finalize)."""
    sc = float(2.0 ** SEXP)
    c2 = (codewords.astype(np.float64) ** 2).sum(1)
    cm_f = (-2.0 * sc * scale[:, None].astype(np.float64) * codewords).T  # [D, K]
    cm8_host = np.ascontiguousarray(
        np.concatenate([cm_f[0:128], cm_f[128:256]], axis=1)
    ).astype(NP_FP8)                                       # [128, 2K]
    # augmove [128, 256]: replicated at partition offsets {0,32,64,96}:
    #   rows p0+j (j<G): scale*2^S in cols [j*K:(j+1)*K], zero elsewhere
    #   row  p0+G:       scale*(c2+256)*2^S in every K-block
    scaleS = (scale.astype(np.float64) * sc).astype(np.float32)
    sc2S = (scale.astype(np.float64) * (c2 + 256.0) * sc).astype(np.float32)
    augmove_host = np.zeros((128, G * K), np.float32)
    for p0 in (0, 32, 64):
        for j in range(G):
            augmove_host[p0 + j, j * K:(j + 1) * K] = scaleS
            augmove_host[p0 + G, j * K:(j + 1) * K] = sc2S
    cw_host = np.ascontiguousarray(codewords.astype(np.float32))
    return {"cm8": cm8_host, "augmove": augmove_host, "cw": cw_host}


def _prep_core(Xcore):
    """Xcore: [NB, D, H, W] fp32 -> xd8 (fp8 DoubleRow layout), xto (bf16)."""
    nb = Xcore.shape[0]
    Xf = Xcore.reshape(nb, D, N)
    # xd8: [nb, 128, 2N]; [b, p, t*N + n] = X[b, t*128+p, n], fp8e4
    xd8 = np.ascontiguousarray(
        Xf.reshape(nb, 2, 128, N).transpose(0, 2, 1, 3).reshape(nb, 128, 2 * N)
    ).astype(NP_FP8)
    # xto: [nb, 128, 72*257]; chunk c holds [X^T rows c*128+p | 1.0]
    XT = np.ascontiguousarray(Xf.transpose(0, 2, 1)).astype(NP_BF16)  # [nb, N, D]
    XTO = np.concatenate([XT, np.ones((nb, N, 1), NP_BF16)], axis=2)  # [nb, N, 257]
    xto = np.ascontiguousarray(
        XTO.reshape(nb, NCHUNK, 128, 257).transpose(0, 2, 1, 3).reshape(nb, 128, NCHUNK * 257)
    )
    return {"xd8": xd8, "xto": xto}


def _build_in_maps(X, codewords, scale):
    shared = _prep_shared(codewords, scale)
    in_maps = []
    for i in range(NC):
        m = dict(shared)
        m.update(_prep_core(X[i * NB:(i + 1) * NB]))
        in_maps.append(m)
    return in_maps


def run(X, codewords, scale, trace=False):
    X = np.asarray(X, np.float32)
    codewords = np.asarray(codewords, np.float32)
    scale = np.asarray(scale, np.float32)
    nc = _get_nc()
    in_maps = _build_in_maps(X, codewords, scale)
    res = run_bass_kernel_spmd(nc, in_maps, list(range(NC)), trace=trace)
    E = np.empty((B, K, D), np.float32)
    for i in range(NC):
        E[i * NB:(i + 1) * NB] = res.results[i]["e"]
    return E, res


def kernel(X, codewords, scale):
    E, _ = run(X, codewords, scale)
    return E
